# revision 4
# baseline (speedup 1.0000x reference)
"""Barycentric interpolation kernel for Trainium2 (8 NeuronCores), v3.

Baseline structure (proven pipeline) with two changes:
  1. ap_gather per GROUP of 2 tiles (num_idxs=12288): amortizes the cost
     model's max(table=10000, ni) charge -> Pool 224us -> 138us.
  2. Weight-broadcast tiles wb_k [128, 4096]: for tiles with ti%2==0 the full
     128-partition image is DMAed from HBM (host-prebuilt); else built via
     masks [2,128] matmul into PSUM + ACT copy (the baseline mechanism).
     This halves the hidden ACT drain wall (168us).
  3. Products are computed in-place into the gathered buffer (saves SBUF).

Layout (as baseline): batches packed in fp16 pairs; partition p holds batches
(2(p%64), +1); partitions 0-63 gather half A of each tile, 64-127 half B.
Output fp16 batch-pair interleaved; host de-interleaves + upcasts.
"""

import numpy as np
from contextlib import ExitStack

B = 128
N = 10000
M = 500000
NCORES = 8
T = 4096                       # target points per tile
H = T // 2                     # points per partition-half
NI = 3 * H                     # gather indices per tile per partition = 6144
TILES_PER_CORE = 16
GROUP_TILES = [1] + [2] * 7 + [1]          # tiles per gather group
GROUPS = len(GROUP_TILES)
M_LOC = T * TILES_PER_CORE     # 65536 padded points per core
M_PAD = M_LOC * NCORES         # 524288

F_SPLIT = 5000                 # f table loads in two halves; tile 0 only
RESTRICT_G0 = True             # needs the first half (host routes points)
WBA_TILE = [ti % 2 == 0 for ti in range(TILES_PER_CORE)]
N_WBA = sum(WBA_TILE)
N_WBB = TILES_PER_CORE - N_WBA
# tiles whose k-sum runs on PE (identity matmuls into PSUM + ACT drain)
PE_KSUM = [False for ti in range(TILES_PER_CORE)]


def _split_drain_waits(nc, mybir):
    """walrus in this toolchain accepts at most one sync-wait on InstDrain;
    move extra waits onto no-ops inserted right after the drain."""
    for f in nc.m.functions:
        for bb in f.blocks:
            insts = list(bb.instructions)
            out, changed = [], False
            for inst in insts:
                out.append(inst)
                si = inst.sync_info
                if (
                    type(inst).__name__ == "InstDrain"
                    and si is not None
                    and si.on_wait is not None
                    and len(si.on_wait) > 1
                ):
                    extras = list(si.on_wait[1:])
                    si.on_wait = [si.on_wait[0]]
                    for w in extras:
                        out.append(
                            mybir.InstNoOp(
                                name=nc.get_next_instruction_name(),
                                engine=inst.engine,
                                sync_info=mybir.SyncInfo(on_wait=[w], on_update=[]),
                                bass_nofuse=True,
                            )
                        )
                    changed = True
            if changed:
                bb.instructions = out


def build_nc():
    import concourse.bacc as bacc
    import concourse.tile as tile
    import concourse.mybir as mybir

    fp16 = mybir.dt.float16
    fp32 = mybir.dt.float32
    i16 = mybir.dt.int16

    nc = bacc.Bacc()
    f_d = nc.declare_dram_parameter("f", [128, N], fp32, isOutput=False)
    idx_d = nc.declare_dram_parameter("idx", [GROUPS, 128, 2 * NI // 16], i16,
                                      isOutput=False)
    wba_d = nc.declare_dram_parameter("wba", [N_WBA, 3, 128, T], fp16,
                                      isOutput=False)
    wpl_d = nc.declare_dram_parameter("wpl", [N_WBB, 3, 16, T // 8], fp16,
                                      isOutput=False)
    masks_d = nc.declare_dram_parameter("masks", [8, 16, 128], fp16,
                                      isOutput=False)

    out_d = nc.declare_dram_parameter("out", [128, M_LOC], fp16, isOutput=True)

    with ExitStack() as ctx:
        tc = ctx.enter_context(tile.TileContext(nc))
        cpool = ctx.enter_context(tc.tile_pool(name="const", bufs=1))
        f_sb = cpool.tile([128, N, 1], fp32)
        masks_sb = [cpool.tile([16, 128], fp16, name=f"masks{c}")
                    for c in range(8)]

        idxp = ctx.enter_context(tc.tile_pool(name="idx", bufs=2))
        # idx DMAs for the first two groups go BEFORE the f-table halves on
        # the serial DMA queue so gather-0 isn't blocked behind them.
        pre_idx = []
        for gi in range(2):
            nig = GROUP_TILES[gi] * NI
            idx_sb = idxp.tile([128, 2 * NI // 16], i16, tag="idx",
                               name=f"idxpre{gi}")
            nc.sync.dma_start(idx_sb[:, :nig // 16], idx_d[gi, :, :nig // 16])
            pre_idx.append(idx_sb)
        nc.sync.dma_start(f_sb[:, :F_SPLIT, 0], f_d[:, :F_SPLIT])
        nc.sync.dma_start(f_sb[:, F_SPLIT:, 0], f_d[:, F_SPLIT:])
        for c in range(8):
            nc.sync.dma_start(masks_sb[c][:], masks_d[c])
        gp = ctx.enter_context(tc.tile_pool(name="g", bufs=2))
        wbp = ctx.enter_context(tc.tile_pool(name="wb", bufs=6))
        wplp = ctx.enter_context(tc.tile_pool(name="wpl", bufs=2))
        op = ctx.enter_context(tc.tile_pool(name="o", bufs=2))
        psp = ctx.enter_context(tc.tile_pool(name="ps", bufs=4, space="PSUM"))

        a_i = b_i = 0
        ti0 = 0
        for gi in range(GROUPS):
            ng = GROUP_TILES[gi]
            nig = ng * NI
            if gi < 2:
                idx_sb = pre_idx[gi]
            else:
                idx_sb = idxp.tile([128, 2 * NI // 16], i16, tag="idx")
                nc.sync.dma_start(idx_sb[:, :nig // 16],
                                  idx_d[gi, :, :nig // 16])
            g3 = gp.tile([128, 2 * NI, 1], fp32, tag="g3")
            if gi == 0 and RESTRICT_G0:
                nc.gpsimd.ap_gather(
                    g3[:, :nig], f_sb[:, :F_SPLIT], idx_sb[:, :nig // 16],
                    channels=128, num_elems=F_SPLIT, d=1, num_idxs=nig,
                )
            else:
                nc.gpsimd.ap_gather(
                    g3[:, :nig], f_sb[:], idx_sb[:, :nig // 16],
                    channels=128, num_elems=N, d=1, num_idxs=nig,
                )
            g16g = g3[:, :, 0].bitcast(fp16)    # [128, 4*NI]

            for tt in range(ng):
                ti = ti0 + tt
                base = tt * 2 * NI              # fp16 offset of tile in group
                outt = op.tile([128, T], fp16, tag="o")
                for k in range(3):
                    ksl = slice(base + k * T, base + (k + 1) * T)
                    wb = wbp.tile([128, T], fp16, tag="wb")
                    if WBA_TILE[ti]:
                        nc.sync.dma_start(wb[:], wba_d[a_i, k])
                    else:
                        wpl = wplp.tile([16, T // 8], fp16, tag="wpl")
                        nc.sync.dma_start(wpl[:], wpl_d[b_i, k])
                        for cc in range(T // 1024):
                            ps = psp.tile([128, 1024], fp32, tag="ps")
                            for c in range(2):
                                ch = cc * 2 + c
                                nc.tensor.matmul(
                                    ps[:, c * 512:(c + 1) * 512],
                                    masks_sb[ch][:],
                                    wpl[:],
                                    start=True, stop=True,
                                )
                            nc.scalar.copy(wb[:, cc * 1024:(cc + 1) * 1024],
                                           ps[:])
                    # in-place product
                    nc.vector.tensor_mul(g16g[:, ksl], g16g[:, ksl], wb[:])
                    if not PE_KSUM[ti]:
                        if k == 1:
                            nc.vector.tensor_add(
                                outt[:], g16g[:, base:base + T],
                                g16g[:, base + T:base + 2 * T])
                        elif k == 2:
                            nc.vector.tensor_add(outt[:], outt[:],
                                                 g16g[:, ksl])
                if PE_KSUM[ti]:
                    for cc in range(T // 1024):
                        ps = psp.tile([128, 1024], fp32, tag="ops")
                        for k in range(3):
                            nc.tensor.matmul(
                                ps[:], ident_sb[:],
                                g16g[:, base + k * T + cc * 1024:
                                        base + k * T + (cc + 1) * 1024],
                                start=(k == 0), stop=(k == 2),
                            )
                        nc.scalar.copy(outt[:, cc * 1024:(cc + 1) * 1024],
                                       ps[:])
                if WBA_TILE[ti]:
                    a_i += 1
                else:
                    b_i += 1
                nc.sync.dma_start(out_d[:, ti * T:(ti + 1) * T], outt[:])
            ti0 += ng

    nc.finalize()
    _split_drain_waits(nc, mybir)
    return nc


# ---------------------------------------------------------------- host side --


def _prep_f(f_values):
    """(128, N) fp32 -> fp32-viewed fp16 batch pairs, duplicated per half."""
    f16 = f_values.astype(np.float16)                    # (128, N)
    pk = np.empty((64, N, 2), np.float16)
    pk[:, :, 0] = f16[0::2]
    pk[:, :, 1] = f16[1::2]
    packed = pk.reshape(64, 2 * N).view(np.float32)      # (64, N)
    return np.ascontiguousarray(np.concatenate([packed, packed], axis=0))


def _wrap16(lst):
    n = lst.shape[0]
    return lst.reshape(n // 16, 16).T


def _prep_core_inputs(ti_core, w_core):
    # per tile: halfA = pts [0, H), halfB = [H, T); k-planar lists
    a = ti_core.reshape(TILES_PER_CORE, 2, H, 3).astype(np.int16)
    lists = a.transpose(0, 1, 3, 2).reshape(TILES_PER_CORE, 2, NI)
    idx = np.zeros((GROUPS, 128, 2 * NI // 16), np.int16)
    t0 = 0
    for g, ng in enumerate(GROUP_TILES):
        la = lists[t0:t0 + ng, 0].reshape(ng * NI)
        lb = lists[t0:t0 + ng, 1].reshape(ng * NI)
        idx[g, :64, :ng * NI // 16] = np.tile(_wrap16(la), (4, 1))
        idx[g, 64:, :ng * NI // 16] = np.tile(_wrap16(lb), (4, 1))
        t0 += ng

    # weights per tile per half per k: dup x2 (pair lanes) -> (T,) rows
    w = w_core.reshape(TILES_PER_CORE, 2, H, 3).astype(np.float16)
    w = w.transpose(0, 1, 3, 2)              # [tile, half, k, H]
    wrow = np.repeat(w, 2, axis=-1)          # [tile, half, k, T]
    wba = np.empty((N_WBA, 3, 128, T), np.float16)
    wpl = np.empty((N_WBB, 3, 16, T // 8), np.float16)
    ai = bi = 0
    for t in range(TILES_PER_CORE):
        if WBA_TILE[t]:
            for k in range(3):
                wba[ai, k, :64] = wrow[t, 0, k]
                wba[ai, k, 64:] = wrow[t, 1, k]
            ai += 1
        else:
            wpl[bi, :, :8] = wrow[t, 0].reshape(3, 8, T // 8)
            wpl[bi, :, 8:] = wrow[t, 1].reshape(3, 8, T // 8)
            bi += 1
    return idx, np.ascontiguousarray(wba), np.ascontiguousarray(wpl)


def _deinterleave(core_out):
    """[128, M_LOC] batch-pair-interleaved -> [128 batches, M_LOC points]."""
    x = core_out.reshape(2, 64, TILES_PER_CORE, H, 2)   # [hf, pp, ti, m, e]
    x = x.transpose(1, 4, 2, 0, 3)                      # [pp, e, ti, hf, m]
    return x.reshape(128, M_LOC)


def kernel(f_values, tri_idx, bary_weights):
    from concourse.bass_utils import run_bass_kernel_spmd

    f_values = np.ascontiguousarray(np.asarray(f_values, dtype=np.float32))
    tri_idx = np.asarray(tri_idx)
    bary_weights = np.asarray(bary_weights)

    ti = np.zeros((M_PAD, 3), np.int32)
    ti[:M] = tri_idx
    w = np.zeros((M_PAD, 3), np.float32)
    w[:M] = bary_weights

    # route points whose 3 indices are all < F_SPLIT to tile 0 of each core:
    # the device gathers tile 0 from the first table half only, so its
    # gather can start before the second f DMA lands.
    perms = []
    for c in range(NCORES):
        tc_ = ti[c * M_LOC:(c + 1) * M_LOC]
        ok = (tc_ < F_SPLIT).all(axis=1)
        sel = np.where(ok)[0]
        assert len(sel) >= T, f"core {c}: only {len(sel)} low-index points"
        sel = sel[:T]
        restmask = np.ones(M_LOC, bool)
        restmask[sel] = False
        perm = np.concatenate([sel, np.where(restmask)[0]])
        perms.append(perm)

    f_h = _prep_f(f_values)
    masks = np.zeros((8, 16, 128), np.float16)
    for c in range(8):
        masks[c, c, :64] = 1.0
        masks[c, 8 + c, 64:] = 1.0
    in_maps = []
    for c in range(NCORES):
        sl = slice(c * M_LOC, (c + 1) * M_LOC)
        idx_h, wba_h, wpl_h = _prep_core_inputs(ti[sl][perms[c]],
                                                w[sl][perms[c]])
        in_maps.append({"f": f_h, "idx": idx_h, "wba": wba_h, "wpl": wpl_h,
                        "masks": masks})

    nc = build_nc()
    res = run_bass_kernel_spmd(nc, in_maps, core_ids=list(range(NCORES)))
    parts = []
    for c in range(NCORES):
        dec = _deinterleave(res.results[c]["out"])
        orig = np.empty_like(dec)
        orig[:, perms[c]] = dec
        parts.append(orig)
    out = np.concatenate(parts, axis=1)
    return out[:, :M].astype(np.float32)


if __name__ == "__main__":
    rng = np.random.default_rng(0)
    f = rng.standard_normal((B, N), dtype=np.float32)
    t_idx = rng.integers(0, N, size=(M, 3)).astype(np.int32)
    bw = rng.random((M, 3), dtype=np.float32)
    bw /= bw.sum(1, keepdims=True)
    got = kernel(f, t_idx, bw)
    exp = np.einsum("bmk,mk->bm", f[:, t_idx], bw)
    err = np.abs(got - exp).max() / np.abs(exp).max()
    print("rel err:", err)


# revision 5
# speedup vs baseline: 1.0223x; 1.0223x over previous
"""Barycentric interpolation kernel for Trainium2 (8 NeuronCores), v3.

Baseline structure (proven pipeline) with two changes:
  1. ap_gather per GROUP of 2 tiles (num_idxs=12288): amortizes the cost
     model's max(table=10000, ni) charge -> Pool 224us -> 138us.
  2. Weight-broadcast tiles wb_k [128, 4096]: for tiles with ti%2==0 the full
     128-partition image is DMAed from HBM (host-prebuilt); else built via
     masks [2,128] matmul into PSUM + ACT copy (the baseline mechanism).
     This halves the hidden ACT drain wall (168us).
  3. Products are computed in-place into the gathered buffer (saves SBUF).

Layout (as baseline): batches packed in fp16 pairs; partition p holds batches
(2(p%64), +1); partitions 0-63 gather half A of each tile, 64-127 half B.
Output fp16 batch-pair interleaved; host de-interleaves + upcasts.
"""

import numpy as np
from contextlib import ExitStack

B = 128
N = 10000
M = 500000
NCORES = 8
T = 4096                       # target points per tile
H = T // 2                     # points per partition-half
NI = 3 * H                     # gather indices per tile per partition = 6144
TILES_PER_CORE = 16
GROUP_TILES = [1] + [2] * 7 + [1]          # tiles per gather group
GROUPS = len(GROUP_TILES)
M_LOC = T * TILES_PER_CORE     # 65536 padded points per core
M_PAD = M_LOC * NCORES         # 524288

F_SPLIT = 5000                 # f table loads in two halves; tile 0 only
RESTRICT_G0 = True             # needs the first half (host routes points)
WBA_TILE = [ti % 2 == 0 for ti in range(TILES_PER_CORE)]
N_WBA = sum(WBA_TILE)
N_WBB = TILES_PER_CORE - N_WBA
# tiles whose k-sum runs on PE (identity matmuls into PSUM + ACT drain)
PE_KSUM = [False for ti in range(TILES_PER_CORE)]


def _split_drain_waits(nc, mybir):
    """walrus in this toolchain accepts at most one sync-wait on InstDrain;
    move extra waits onto no-ops inserted right after the drain."""
    for f in nc.m.functions:
        for bb in f.blocks:
            insts = list(bb.instructions)
            out, changed = [], False
            for inst in insts:
                out.append(inst)
                si = inst.sync_info
                if (
                    type(inst).__name__ == "InstDrain"
                    and si is not None
                    and si.on_wait is not None
                    and len(si.on_wait) > 1
                ):
                    extras = list(si.on_wait[1:])
                    si.on_wait = [si.on_wait[0]]
                    for w in extras:
                        out.append(
                            mybir.InstNoOp(
                                name=nc.get_next_instruction_name(),
                                engine=inst.engine,
                                sync_info=mybir.SyncInfo(on_wait=[w], on_update=[]),
                                bass_nofuse=True,
                            )
                        )
                    changed = True
            if changed:
                bb.instructions = out


def build_nc():
    import concourse.bacc as bacc
    import concourse.tile as tile
    import concourse.mybir as mybir

    fp16 = mybir.dt.float16
    fp32 = mybir.dt.float32
    i16 = mybir.dt.int16

    nc = bacc.Bacc()
    f_d = nc.declare_dram_parameter("f", [128, N], fp32, isOutput=False)
    idx_d = nc.declare_dram_parameter("idx", [GROUPS, 128, 2 * NI // 16], i16,
                                      isOutput=False)
    wba_d = nc.declare_dram_parameter("wba", [N_WBA, 3, 128, T], fp16,
                                      isOutput=False)
    wpl_d = nc.declare_dram_parameter("wpl", [N_WBB, 3, 16, T // 8], fp16,
                                      isOutput=False)
    masks_d = nc.declare_dram_parameter("masks", [8, 16, 128], fp16,
                                      isOutput=False)

    out_d = nc.declare_dram_parameter("out", [128, M_LOC], fp16, isOutput=True)

    with ExitStack() as ctx:
        tc = ctx.enter_context(tile.TileContext(nc))
        cpool = ctx.enter_context(tc.tile_pool(name="const", bufs=1))
        f_sb = cpool.tile([128, N, 1], fp32)
        masks_sb = [cpool.tile([16, 128], fp16, name=f"masks{c}")
                    for c in range(8)]

        idxp = ctx.enter_context(tc.tile_pool(name="idx", bufs=2))
        # idx DMAs for the first two groups go BEFORE the f-table halves on
        # the serial DMA queue so gather-0 isn't blocked behind them.
        pre_idx = []
        for gi in range(2):
            nig = GROUP_TILES[gi] * NI
            idx_sb = idxp.tile([128, 2 * NI // 16], i16, tag="idx",
                               name=f"idxpre{gi}")
            nc.sync.dma_start(idx_sb[:, :nig // 16], idx_d[gi, :, :nig // 16])
            pre_idx.append(idx_sb)
        nc.sync.dma_start(f_sb[:, :F_SPLIT, 0], f_d[:, :F_SPLIT])
        nc.sync.dma_start(f_sb[:, F_SPLIT:, 0], f_d[:, F_SPLIT:])
        for c in range(8):
            nc.sync.dma_start(masks_sb[c][:], masks_d[c])
        gp = ctx.enter_context(tc.tile_pool(name="g", bufs=2))
        wbp = ctx.enter_context(tc.tile_pool(name="wb", bufs=4))
        wplp = ctx.enter_context(tc.tile_pool(name="wpl", bufs=2))
        op = ctx.enter_context(tc.tile_pool(name="o", bufs=2))
        sp = ctx.enter_context(tc.tile_pool(name="s", bufs=2))
        psp = ctx.enter_context(tc.tile_pool(name="ps", bufs=4, space="PSUM"))

        a_i = b_i = 0
        ti0 = 0
        for gi in range(GROUPS):
            ng = GROUP_TILES[gi]
            nig = ng * NI
            if gi < 2:
                idx_sb = pre_idx[gi]
            else:
                idx_sb = idxp.tile([128, 2 * NI // 16], i16, tag="idx")
                nc.sync.dma_start(idx_sb[:, :nig // 16],
                                  idx_d[gi, :, :nig // 16])
            g3 = gp.tile([128, 2 * NI, 1], fp32, tag="g3")
            if gi == 0 and RESTRICT_G0:
                nc.gpsimd.ap_gather(
                    g3[:, :nig], f_sb[:, :F_SPLIT], idx_sb[:, :nig // 16],
                    channels=128, num_elems=F_SPLIT, d=1, num_idxs=nig,
                )
            else:
                nc.gpsimd.ap_gather(
                    g3[:, :nig], f_sb[:], idx_sb[:, :nig // 16],
                    channels=128, num_elems=N, d=1, num_idxs=nig,
                )
            g16g = g3[:, :, 0].bitcast(fp16)    # [128, 4*NI]

            for tt in range(ng):
                ti = ti0 + tt
                base = tt * 2 * NI              # fp16 offset of tile in group
                outt = op.tile([128, T], fp16, tag="o")
                for k in range(3):
                    ksl = slice(base + k * T, base + (k + 1) * T)
                    wb = wbp.tile([128, T], fp16, tag="wb")
                    if WBA_TILE[ti]:
                        nc.sync.dma_start(wb[:], wba_d[a_i, k])
                    else:
                        wpl = wplp.tile([16, T // 8], fp16, tag="wpl")
                        nc.sync.dma_start(wpl[:], wpl_d[b_i, k])
                        for cc in range(T // 1024):
                            ps = psp.tile([128, 1024], fp32, tag="ps")
                            for c in range(2):
                                ch = cc * 2 + c
                                nc.tensor.matmul(
                                    ps[:, c * 512:(c + 1) * 512],
                                    masks_sb[ch][:],
                                    wpl[:],
                                    start=True, stop=True,
                                )
                            nc.scalar.copy(wb[:, cc * 1024:(cc + 1) * 1024],
                                           ps[:])
                    # in-place product
                    nc.vector.tensor_mul(g16g[:, ksl], g16g[:, ksl], wb[:])
                    if not PE_KSUM[ti]:
                        if k == 1:
                            nc.vector.tensor_add(
                                outt[:], g16g[:, base:base + T],
                                g16g[:, base + T:base + 2 * T])
                        elif k == 2:
                            nc.vector.tensor_add(outt[:], outt[:],
                                                 g16g[:, ksl])
                if PE_KSUM[ti]:
                    for cc in range(T // 1024):
                        ps = psp.tile([128, 1024], fp32, tag="ops")
                        for k in range(3):
                            nc.tensor.matmul(
                                ps[:], ident_sb[:],
                                g16g[:, base + k * T + cc * 1024:
                                        base + k * T + (cc + 1) * 1024],
                                start=(k == 0), stop=(k == 2),
                            )
                        nc.scalar.copy(outt[:, cc * 1024:(cc + 1) * 1024],
                                       ps[:])
                if WBA_TILE[ti]:
                    a_i += 1
                else:
                    b_i += 1
                nc.sync.dma_start(out_d[:, ti * T:(ti + 1) * T], outt[:])
            ti0 += ng

    nc.finalize()
    _split_drain_waits(nc, mybir)
    return nc


# ---------------------------------------------------------------- host side --


def _prep_f(f_values):
    """(128, N) fp32 -> fp32-viewed fp16 batch pairs, duplicated per half."""
    f16 = f_values.astype(np.float16)                    # (128, N)
    pk = np.empty((64, N, 2), np.float16)
    pk[:, :, 0] = f16[0::2]
    pk[:, :, 1] = f16[1::2]
    packed = pk.reshape(64, 2 * N).view(np.float32)      # (64, N)
    return np.ascontiguousarray(np.concatenate([packed, packed], axis=0))


def _wrap16(lst):
    n = lst.shape[0]
    return lst.reshape(n // 16, 16).T


def _prep_core_inputs(ti_core, w_core):
    # per tile: halfA = pts [0, H), halfB = [H, T); k-planar lists
    a = ti_core.reshape(TILES_PER_CORE, 2, H, 3).astype(np.int16)
    lists = a.transpose(0, 1, 3, 2).reshape(TILES_PER_CORE, 2, NI)
    idx = np.zeros((GROUPS, 128, 2 * NI // 16), np.int16)
    t0 = 0
    for g, ng in enumerate(GROUP_TILES):
        la = lists[t0:t0 + ng, 0].reshape(ng * NI)
        lb = lists[t0:t0 + ng, 1].reshape(ng * NI)
        idx[g, :64, :ng * NI // 16] = np.tile(_wrap16(la), (4, 1))
        idx[g, 64:, :ng * NI // 16] = np.tile(_wrap16(lb), (4, 1))
        t0 += ng

    # weights per tile per half per k: dup x2 (pair lanes) -> (T,) rows
    w = w_core.reshape(TILES_PER_CORE, 2, H, 3).astype(np.float16)
    w = w.transpose(0, 1, 3, 2)              # [tile, half, k, H]
    wrow = np.repeat(w, 2, axis=-1)          # [tile, half, k, T]
    wba = np.empty((N_WBA, 3, 128, T), np.float16)
    wpl = np.empty((N_WBB, 3, 16, T // 8), np.float16)
    ai = bi = 0
    for t in range(TILES_PER_CORE):
        if WBA_TILE[t]:
            for k in range(3):
                wba[ai, k, :64] = wrow[t, 0, k]
                wba[ai, k, 64:] = wrow[t, 1, k]
            ai += 1
        else:
            wpl[bi, :, :8] = wrow[t, 0].reshape(3, 8, T // 8)
            wpl[bi, :, 8:] = wrow[t, 1].reshape(3, 8, T // 8)
            bi += 1
    return idx, np.ascontiguousarray(wba), np.ascontiguousarray(wpl)


def _deinterleave(core_out):
    """[128, M_LOC] batch-pair-interleaved -> [128 batches, M_LOC points]."""
    x = core_out.reshape(2, 64, TILES_PER_CORE, H, 2)   # [hf, pp, ti, m, e]
    x = x.transpose(1, 4, 2, 0, 3)                      # [pp, e, ti, hf, m]
    return x.reshape(128, M_LOC)


def kernel(f_values, tri_idx, bary_weights):
    from concourse.bass_utils import run_bass_kernel_spmd

    f_values = np.ascontiguousarray(np.asarray(f_values, dtype=np.float32))
    tri_idx = np.asarray(tri_idx)
    bary_weights = np.asarray(bary_weights)

    ti = np.zeros((M_PAD, 3), np.int32)
    ti[:M] = tri_idx
    w = np.zeros((M_PAD, 3), np.float32)
    w[:M] = bary_weights

    # route points whose 3 indices are all < F_SPLIT to tile 0 of each core:
    # the device gathers tile 0 from the first table half only, so its
    # gather can start before the second f DMA lands.
    perms = []
    for c in range(NCORES):
        tc_ = ti[c * M_LOC:(c + 1) * M_LOC]
        ok = (tc_ < F_SPLIT).all(axis=1)
        sel = np.where(ok)[0]
        assert len(sel) >= T, f"core {c}: only {len(sel)} low-index points"
        sel = sel[:T]
        restmask = np.ones(M_LOC, bool)
        restmask[sel] = False
        perm = np.concatenate([sel, np.where(restmask)[0]])
        perms.append(perm)

    f_h = _prep_f(f_values)
    masks = np.zeros((8, 16, 128), np.float16)
    for c in range(8):
        masks[c, c, :64] = 1.0
        masks[c, 8 + c, 64:] = 1.0
    in_maps = []
    for c in range(NCORES):
        sl = slice(c * M_LOC, (c + 1) * M_LOC)
        idx_h, wba_h, wpl_h = _prep_core_inputs(ti[sl][perms[c]],
                                                w[sl][perms[c]])
        in_maps.append({"f": f_h, "idx": idx_h, "wba": wba_h, "wpl": wpl_h,
                        "masks": masks})

    nc = build_nc()
    res = run_bass_kernel_spmd(nc, in_maps, core_ids=list(range(NCORES)))
    parts = []
    for c in range(NCORES):
        dec = _deinterleave(res.results[c]["out"])
        orig = np.empty_like(dec)
        orig[:, perms[c]] = dec
        parts.append(orig)
    out = np.concatenate(parts, axis=1)
    return out[:, :M].astype(np.float32)


if __name__ == "__main__":
    rng = np.random.default_rng(0)
    f = rng.standard_normal((B, N), dtype=np.float32)
    t_idx = rng.integers(0, N, size=(M, 3)).astype(np.int32)
    bw = rng.random((M, 3), dtype=np.float32)
    bw /= bw.sum(1, keepdims=True)
    got = kernel(f, t_idx, bw)
    exp = np.einsum("bmk,mk->bm", f[:, t_idx], bw)
    err = np.abs(got - exp).max() / np.abs(exp).max()
    print("rel err:", err)


# revision 6
# speedup vs baseline: 1.0291x; 1.0067x over previous
"""Barycentric interpolation kernel for Trainium2 (8 NeuronCores), v3.

Baseline structure (proven pipeline) with two changes:
  1. ap_gather per GROUP of 2 tiles (num_idxs=12288): amortizes the cost
     model's max(table=10000, ni) charge -> Pool 224us -> 138us.
  2. Weight-broadcast tiles wb_k [128, 4096]: for tiles with ti%2==0 the full
     128-partition image is DMAed from HBM (host-prebuilt); else built via
     masks [2,128] matmul into PSUM + ACT copy (the baseline mechanism).
     This halves the hidden ACT drain wall (168us).
  3. Products are computed in-place into the gathered buffer (saves SBUF).

Layout (as baseline): batches packed in fp16 pairs; partition p holds batches
(2(p%64), +1); partitions 0-63 gather half A of each tile, 64-127 half B.
Output fp16 batch-pair interleaved; host de-interleaves + upcasts.
"""

import numpy as np
from contextlib import ExitStack

B = 128
N = 10000
M = 500000
NCORES = 8
T = 4096                       # target points per tile
H = T // 2                     # points per partition-half
NI = 3 * H                     # gather indices per tile per partition = 6144
TILES_PER_CORE = 16
GROUP_TILES = [1] + [2] * 7 + [1]          # tiles per gather group
GROUPS = len(GROUP_TILES)
M_LOC = T * TILES_PER_CORE     # 65536 padded points per core
M_PAD = M_LOC * NCORES         # 524288

F_SPLIT = 5000                 # f table loads in two halves; tile 0 only
RESTRICT_G0 = True             # needs the first half (host routes points)
WBA_TILE = [ti % 2 == 0 for ti in range(TILES_PER_CORE)]
N_WBA = sum(WBA_TILE)
N_WBB = TILES_PER_CORE - N_WBA
# tiles whose k-sum runs on PE (identity matmuls into PSUM + ACT drain)
PE_KSUM = [False for ti in range(TILES_PER_CORE)]


def _split_drain_waits(nc, mybir):
    """walrus in this toolchain accepts at most one sync-wait on InstDrain;
    move extra waits onto no-ops inserted right after the drain."""
    for f in nc.m.functions:
        for bb in f.blocks:
            insts = list(bb.instructions)
            out, changed = [], False
            for inst in insts:
                out.append(inst)
                si = inst.sync_info
                if (
                    type(inst).__name__ == "InstDrain"
                    and si is not None
                    and si.on_wait is not None
                    and len(si.on_wait) > 1
                ):
                    extras = list(si.on_wait[1:])
                    si.on_wait = [si.on_wait[0]]
                    for w in extras:
                        out.append(
                            mybir.InstNoOp(
                                name=nc.get_next_instruction_name(),
                                engine=inst.engine,
                                sync_info=mybir.SyncInfo(on_wait=[w], on_update=[]),
                                bass_nofuse=True,
                            )
                        )
                    changed = True
            if changed:
                bb.instructions = out


def build_nc():
    import concourse.bacc as bacc
    import concourse.tile as tile
    import concourse.mybir as mybir

    fp16 = mybir.dt.float16
    fp32 = mybir.dt.float32
    i16 = mybir.dt.int16

    nc = bacc.Bacc()
    f_d = nc.declare_dram_parameter("f", [128, N], fp32, isOutput=False)
    idx_d = nc.declare_dram_parameter("idx", [GROUPS, 128, 2 * NI // 16], i16,
                                      isOutput=False)
    wba_d = nc.declare_dram_parameter("wba", [N_WBA, 3, 128, T], fp16,
                                      isOutput=False)
    wpl_d = nc.declare_dram_parameter("wpl", [N_WBB, 3, 16, T // 8], fp16,
                                      isOutput=False)
    masks_d = nc.declare_dram_parameter("masks", [8, 16, 128], fp16,
                                      isOutput=False)

    out_d = nc.declare_dram_parameter("out", [128, M_LOC], fp16, isOutput=True)

    with ExitStack() as ctx:
        tc = ctx.enter_context(tile.TileContext(nc))
        cpool = ctx.enter_context(tc.tile_pool(name="const", bufs=1))
        f_sb = cpool.tile([128, N, 1], fp32)
        masks_sb = [cpool.tile([16, 128], fp16, name=f"masks{c}")
                    for c in range(8)]

        idxp = ctx.enter_context(tc.tile_pool(name="idx", bufs=2))
        # idx DMAs for the first two groups go BEFORE the f-table halves on
        # the serial DMA queue so gather-0 isn't blocked behind them.
        pre_idx = []
        for gi in range(2):
            nig = GROUP_TILES[gi] * NI
            idx_sb = idxp.tile([128, 2 * NI // 16], i16, tag="idx",
                               name=f"idxpre{gi}")
            nc.sync.dma_start(idx_sb[:, :nig // 16], idx_d[gi, :, :nig // 16])
            pre_idx.append(idx_sb)
        nc.sync.dma_start(f_sb[:, :F_SPLIT, 0], f_d[:, :F_SPLIT])
        nc.sync.dma_start(f_sb[:, F_SPLIT:, 0], f_d[:, F_SPLIT:])
        for c in range(8):
            nc.sync.dma_start(masks_sb[c][:], masks_d[c])
        gp = ctx.enter_context(tc.tile_pool(name="g", bufs=2))
        wbp = ctx.enter_context(tc.tile_pool(name="wb", bufs=4))
        wplp = ctx.enter_context(tc.tile_pool(name="wpl", bufs=2))
        op = ctx.enter_context(tc.tile_pool(name="o", bufs=2))
        sp = ctx.enter_context(tc.tile_pool(name="s", bufs=2))
        psp = ctx.enter_context(tc.tile_pool(name="ps", bufs=4, space="PSUM"))

        a_i = b_i = 0
        ti0 = 0
        for gi in range(GROUPS):
            ng = GROUP_TILES[gi]
            nig = ng * NI
            if gi < 2:
                idx_sb = pre_idx[gi]
            else:
                idx_sb = idxp.tile([128, 2 * NI // 16], i16, tag="idx")
                nc.sync.dma_start(idx_sb[:, :nig // 16],
                                  idx_d[gi, :, :nig // 16])
            g3 = gp.tile([128, 2 * NI, 1], fp32, tag="g3")
            if gi == 0 and RESTRICT_G0:
                nc.gpsimd.ap_gather(
                    g3[:, :nig], f_sb[:, :F_SPLIT], idx_sb[:, :nig // 16],
                    channels=128, num_elems=F_SPLIT, d=1, num_idxs=nig,
                )
            else:
                nc.gpsimd.ap_gather(
                    g3[:, :nig], f_sb[:], idx_sb[:, :nig // 16],
                    channels=128, num_elems=N, d=1, num_idxs=nig,
                )
            g16g = g3[:, :, 0].bitcast(fp16)    # [128, 4*NI]

            for tt in range(ng):
                ti = ti0 + tt
                base = tt * 2 * NI              # fp16 offset of tile in group
                outt = op.tile([128, T], fp16, tag="o")
                for k in range(3):
                    ksl = slice(base + k * T, base + (k + 1) * T)
                    wb = wbp.tile([128, T], fp16, tag="wb")
                    if WBA_TILE[ti]:
                        nc.sync.dma_start(wb[:], wba_d[a_i, k])
                    else:
                        wpl = wplp.tile([16, T // 8], fp16, tag="wpl")
                        nc.sync.dma_start(wpl[:], wpl_d[b_i, k])
                        for cc in range(T // 1024):
                            ps = psp.tile([128, 1024], fp32, tag="ps")
                            for c in range(2):
                                ch = cc * 2 + c
                                nc.tensor.matmul(
                                    ps[:, c * 512:(c + 1) * 512],
                                    masks_sb[ch][:],
                                    wpl[:],
                                    start=True, stop=True,
                                )
                            nc.scalar.copy(wb[:, cc * 1024:(cc + 1) * 1024],
                                           ps[:])
                    # in-place product; last tile runs in halves so its
                    # first out-DMA half overlaps the rest of the compute
                    nh = 4 if ti == TILES_PER_CORE - 1 else 1
                    hw_ = T // nh
                    for hh in range(nh):
                        hsl = slice(hh * hw_, (hh + 1) * hw_)
                        kslh = slice(ksl.start + hh * hw_,
                                     ksl.start + (hh + 1) * hw_)
                        nc.vector.tensor_mul(g16g[:, kslh], g16g[:, kslh],
                                             wb[:, hsl])
                        if k == 1:
                            nc.vector.tensor_add(
                                outt[:, hsl],
                                g16g[:, base + hh * hw_:
                                        base + (hh + 1) * hw_],
                                g16g[:, base + T + hh * hw_:
                                        base + T + (hh + 1) * hw_])
                        elif k == 2:
                            nc.vector.tensor_add(outt[:, hsl],
                                                 outt[:, hsl],
                                                 g16g[:, kslh])
                            if nh > 1:
                                nc.sync.dma_start(
                                    out_d[:, ti * T + hh * hw_:
                                             ti * T + (hh + 1) * hw_],
                                    outt[:, hsl])
                if PE_KSUM[ti]:
                    for cc in range(T // 1024):
                        ps = psp.tile([128, 1024], fp32, tag="ops")
                        for k in range(3):
                            nc.tensor.matmul(
                                ps[:], ident_sb[:],
                                g16g[:, base + k * T + cc * 1024:
                                        base + k * T + (cc + 1) * 1024],
                                start=(k == 0), stop=(k == 2),
                            )
                        nc.scalar.copy(outt[:, cc * 1024:(cc + 1) * 1024],
                                       ps[:])
                if WBA_TILE[ti]:
                    a_i += 1
                else:
                    b_i += 1
                if ti != TILES_PER_CORE - 1:
                    nc.sync.dma_start(out_d[:, ti * T:(ti + 1) * T],
                                      outt[:])
            ti0 += ng

    nc.finalize()
    _split_drain_waits(nc, mybir)
    return nc


# ---------------------------------------------------------------- host side --


def _prep_f(f_values):
    """(128, N) fp32 -> fp32-viewed fp16 batch pairs, duplicated per half."""
    f16 = f_values.astype(np.float16)                    # (128, N)
    pk = np.empty((64, N, 2), np.float16)
    pk[:, :, 0] = f16[0::2]
    pk[:, :, 1] = f16[1::2]
    packed = pk.reshape(64, 2 * N).view(np.float32)      # (64, N)
    return np.ascontiguousarray(np.concatenate([packed, packed], axis=0))


def _wrap16(lst):
    n = lst.shape[0]
    return lst.reshape(n // 16, 16).T


def _prep_core_inputs(ti_core, w_core):
    # per tile: halfA = pts [0, H), halfB = [H, T); k-planar lists
    a = ti_core.reshape(TILES_PER_CORE, 2, H, 3).astype(np.int16)
    lists = a.transpose(0, 1, 3, 2).reshape(TILES_PER_CORE, 2, NI)
    idx = np.zeros((GROUPS, 128, 2 * NI // 16), np.int16)
    t0 = 0
    for g, ng in enumerate(GROUP_TILES):
        la = lists[t0:t0 + ng, 0].reshape(ng * NI)
        lb = lists[t0:t0 + ng, 1].reshape(ng * NI)
        idx[g, :64, :ng * NI // 16] = np.tile(_wrap16(la), (4, 1))
        idx[g, 64:, :ng * NI // 16] = np.tile(_wrap16(lb), (4, 1))
        t0 += ng

    # weights per tile per half per k: dup x2 (pair lanes) -> (T,) rows
    w = w_core.reshape(TILES_PER_CORE, 2, H, 3).astype(np.float16)
    w = w.transpose(0, 1, 3, 2)              # [tile, half, k, H]
    wrow = np.repeat(w, 2, axis=-1)          # [tile, half, k, T]
    wba = np.empty((N_WBA, 3, 128, T), np.float16)
    wpl = np.empty((N_WBB, 3, 16, T // 8), np.float16)
    ai = bi = 0
    for t in range(TILES_PER_CORE):
        if WBA_TILE[t]:
            for k in range(3):
                wba[ai, k, :64] = wrow[t, 0, k]
                wba[ai, k, 64:] = wrow[t, 1, k]
            ai += 1
        else:
            wpl[bi, :, :8] = wrow[t, 0].reshape(3, 8, T // 8)
            wpl[bi, :, 8:] = wrow[t, 1].reshape(3, 8, T // 8)
            bi += 1
    return idx, np.ascontiguousarray(wba), np.ascontiguousarray(wpl)


def _deinterleave(core_out):
    """[128, M_LOC] batch-pair-interleaved -> [128 batches, M_LOC points]."""
    x = core_out.reshape(2, 64, TILES_PER_CORE, H, 2)   # [hf, pp, ti, m, e]
    x = x.transpose(1, 4, 2, 0, 3)                      # [pp, e, ti, hf, m]
    return x.reshape(128, M_LOC)


def kernel(f_values, tri_idx, bary_weights):
    from concourse.bass_utils import run_bass_kernel_spmd

    f_values = np.ascontiguousarray(np.asarray(f_values, dtype=np.float32))
    tri_idx = np.asarray(tri_idx)
    bary_weights = np.asarray(bary_weights)

    ti = np.zeros((M_PAD, 3), np.int32)
    ti[:M] = tri_idx
    w = np.zeros((M_PAD, 3), np.float32)
    w[:M] = bary_weights

    # route points whose 3 indices are all < F_SPLIT to tile 0 of each core:
    # the device gathers tile 0 from the first table half only, so its
    # gather can start before the second f DMA lands.
    perms = []
    for c in range(NCORES):
        tc_ = ti[c * M_LOC:(c + 1) * M_LOC]
        ok = (tc_ < F_SPLIT).all(axis=1)
        sel = np.where(ok)[0]
        assert len(sel) >= T, f"core {c}: only {len(sel)} low-index points"
        sel = sel[:T]
        restmask = np.ones(M_LOC, bool)
        restmask[sel] = False
        perm = np.concatenate([sel, np.where(restmask)[0]])
        perms.append(perm)

    f_h = _prep_f(f_values)
    masks = np.zeros((8, 16, 128), np.float16)
    for c in range(8):
        masks[c, c, :64] = 1.0
        masks[c, 8 + c, 64:] = 1.0
    in_maps = []
    for c in range(NCORES):
        sl = slice(c * M_LOC, (c + 1) * M_LOC)
        idx_h, wba_h, wpl_h = _prep_core_inputs(ti[sl][perms[c]],
                                                w[sl][perms[c]])
        in_maps.append({"f": f_h, "idx": idx_h, "wba": wba_h, "wpl": wpl_h,
                        "masks": masks})

    nc = build_nc()
    res = run_bass_kernel_spmd(nc, in_maps, core_ids=list(range(NCORES)))
    parts = []
    for c in range(NCORES):
        dec = _deinterleave(res.results[c]["out"])
        orig = np.empty_like(dec)
        orig[:, perms[c]] = dec
        parts.append(orig)
    out = np.concatenate(parts, axis=1)
    return out[:, :M].astype(np.float32)


if __name__ == "__main__":
    rng = np.random.default_rng(0)
    f = rng.standard_normal((B, N), dtype=np.float32)
    t_idx = rng.integers(0, N, size=(M, 3)).astype(np.int32)
    bw = rng.random((M, 3), dtype=np.float32)
    bw /= bw.sum(1, keepdims=True)
    got = kernel(f, t_idx, bw)
    exp = np.einsum("bmk,mk->bm", f[:, t_idx], bw)
    err = np.abs(got - exp).max() / np.abs(exp).max()
    print("rel err:", err)
